# revision 21
# baseline (speedup 1.0000x reference)
"""Bass/Tile TRN2 kernel for additive-attention pooling.

Math per sample s:
    e = tanh(x[s] @ W + b)          # (T, 1)
    a = softmax(e, axis=0)          # over T
    y[s] = sum_t a[t] * x[s, t, :]  # (U,)

tanh is bounded in (-1, 1), so softmax needs no max-subtraction:
    p = exp(e);  y[s] = (sum_t p[t] x[s,t]) / (sum_t p[t])

Sharding: data-parallel over batch across 8 NeuronCores (32 samples each).

Per-core dataflow (v7). x is streamed once, one whole sample (4 MiB)
per DMA on a single HWDGE queue (sequential HBM addresses; splitting
across two queue paths measured 17% slower). Layout is q-packed
(t = p*16 + q) so partition p receives 32 KiB contiguous HBM per
sample; all timesteps are symmetric under the time-reduction so the
packing order is free.

The per-timestep dot e_t = x_t . W is the DVE/ACT-shared workload (the
fp32 fused multiply-accumulate on DVE alone slightly exceeds the DMA
rate — 'ridge' regime):
  - 5 of 8 q-slices per half-sample: DVE scalar_tensor_tensor (fused
    multiply + free-dim accumulate, 1 elem/lane/cyc)
  - 3 of 8: DVE does one fused fp32->bf16 multiply [128,1536], ACT
    reduces each 512-slice via ACTIVATE(Copy, accum_out)
  - the LAST sample runs all-DVE so its drain chain is short (the
    TT-mult -> ACT-reduce chain otherwise serializes the kernel tail)
Then ACT tanh/exp, and per q-slice an accumulating fp32 matmul
(K=128, M=1, N=512) into col group q%4 of a PSUM bank — 4 concurrent
streams via separate XBUSes. Per sample: partials combined via a
selector matmul, denominator via ones-matmul, DVE reciprocal, ACT
scaled copy, DMA out.
"""

from contextlib import ExitStack

import numpy as np

B, T, U = 256, 2048, 512
N_CORES = 8
B_LOC = B // N_CORES
P = 128

_BUILD_CACHE = {}


def _emit(ctx, tc, x, W, b, y, xbufs):
    from concourse import mybir

    nc = tc.nc
    f32 = mybir.dt.float32
    bf16 = mybir.dt.bfloat16
    Alu = mybir.AluOpType
    Act = mybir.ActivationFunctionType

    b_loc, t_len, u = x.shape
    QT = t_len // P              # timesteps per partition per sample (16)
    NC_CHUNK = 2                 # compute chunks per sample
    Q = QT // NC_CHUNK           # q-slices per compute chunk (8)
    N_ACT = 3                    # q-slices per chunk reduced on ACT
    Q_DVE = Q - N_ACT

    const = ctx.enter_context(tc.tile_pool(name="const", bufs=1))
    xp = ctx.enter_context(tc.tile_pool(name="xp", bufs=xbufs))
    scr_p = ctx.enter_context(tc.tile_pool(name="scr", bufs=2))
    scrb_p = ctx.enter_context(tc.tile_pool(name="scrb", bufs=3))
    dump_p = ctx.enter_context(tc.tile_pool(name="dump", bufs=2))
    ep = ctx.enter_context(tc.tile_pool(name="ep", bufs=4))
    sp = ctx.enter_context(tc.tile_pool(name="sp", bufs=6))
    op = ctx.enter_context(tc.tile_pool(name="op", bufs=4))
    ps_wb = ctx.enter_context(tc.tile_pool(name="ps_wb", bufs=1, space="PSUM"))
    ps_w = ctx.enter_context(tc.tile_pool(name="ps_w", bufs=4, space="PSUM"))
    ps_s = ctx.enter_context(tc.tile_pool(name="ps_s", bufs=2, space="PSUM"))

    # ---- constants ----
    # W as a [1, U] row, broadcast to all 128 partitions via a K=1 matmul.
    w_row = const.tile([1, u], f32)
    nc.sync.dma_start(w_row[:], W.rearrange("u o -> o u"))
    ones_row = const.tile([1, P], f32)
    nc.vector.memset(ones_row[:], 1.0)
    ones_col = const.tile([P, 1], f32)
    nc.vector.memset(ones_col[:], 1.0)
    wb_ps = ps_wb.tile([P, u], f32, tag="wb_ps")
    nc.tensor.matmul(wb_ps[:], ones_row[:], w_row[:], start=True, stop=True)
    Wb = const.tile([P, u], f32)
    nc.vector.tensor_copy(Wb[:], wb_ps[:])
    # W replicated across the ACT-reduced slice group for the fused multiply
    Wb3 = const.tile([P, N_ACT * u], f32)
    for j in range(N_ACT):
        nc.vector.tensor_copy(Wb3[:, j * u:(j + 1) * u], wb_ps[:])
    # selector: 1.0 at partitions {0,32,64,96} — sums the 4 col-group partials
    sel = const.tile([P, 1], f32)
    nc.vector.memset(sel[:], 0.0)
    for j in range(4):
        nc.vector.memset(sel[32 * j:32 * j + 1, :], 1.0)
    # b rearranged to [partition, q] matching the q-packed x layout
    bt = const.tile([P, QT], f32)
    nc.sync.dma_start(bt[:], b.rearrange("(p q) o -> p (q o)", p=P, q=QT))

    xr = x.rearrange("s (p q) u -> s p (q u)", p=P, q=QT)

    def emit_epilogue(se, wsum, rs):
        # denominator: rstot[p] = sum_c rs[p,c] (ACT accum); d = rstot.T @ ones
        rsd = sp.tile([P, NC_CHUNK], f32, tag="rsd")
        rstot = sp.tile([P, 1], f32, tag="rstot")
        nc.scalar.activation(rsd[:], rs[:], Act.Copy, accum_out=rstot[:])
        s_ps = ps_s.tile([1, 1], f32)
        nc.tensor.matmul(s_ps[:], rstot[:], ones_col[:], start=True, stop=True)
        inv = sp.tile([1, 1], f32, tag="inv")
        nc.vector.reciprocal(inv[:], s_ps[:])

        # combine the 4 partial rows: copy bank to SBUF, then sel.T @ rows
        wsb = op.tile([P, u], f32, tag="wsb")
        nc.scalar.activation(wsb[:], wsum[:], Act.Copy)
        yrow = ps_wb.tile([1, u], f32, tag="yrow")
        nc.tensor.matmul(yrow[:], sel[:], wsb[:], start=True, stop=True)

        orow = op.tile([1, u], f32, tag="orow")
        nc.scalar.activation(orow[:], yrow[:], Act.Copy, scale=inv[:])
        nc.sync.dma_start(y[se:se + 1, :], orow[:])

    # Epilogues are emitted one sample late (after the NEXT sample's first
    # compute chunk): ACT/DVE are strict-FIFO, so an epilogue op waiting on
    # the PE at the queue head would head-of-line block the next sample's
    # reduce/exp work. Deferring it means its sems are satisfied on arrival.
    pending = None
    for s in range(b_loc):
        wsum = ps_w.tile([P, u], f32, tag="wsum")
        nc.scalar.memzero(wsum[:])
        rs = sp.tile([P, NC_CHUNK], f32, tag="rs")
        # whole sample in one DMA: partition p holds 16 contiguous HBM rows
        xt = xp.tile([P, QT * u], f32)
        nc.sync.dma_start(xt[:], xr[s])
        # last sample: all slices on DVE so the kernel tail has no long
        # cross-engine reduce chain to drain
        n_act = N_ACT if s < b_loc - 1 else 0
        q_dve = Q - n_act

        for c in range(NC_CHUNK):
            base = c * Q
            e_c = ep.tile([P, Q], f32, tag="e_c")
            # DVE: fused multiply + accumulate for the first q_dve slices
            for qi in range(q_dve):
                q = base + qi
                scr = scr_p.tile([P, u], f32)
                nc.vector.scalar_tensor_tensor(
                    out=scr[:],
                    in0=xt[:, q * u:(q + 1) * u],
                    scalar=1.0,
                    in1=Wb[:],
                    op0=Alu.mult,
                    op1=Alu.mult,
                    accum_out=e_c[:, qi:qi + 1],
                )
            if n_act:
                # DVE: one fused multiply for the ACT-reduced slices
                scrb = scrb_p.tile([P, n_act * u], bf16)
                nc.vector.tensor_mul(
                    scrb[:], xt[:, (base + q_dve) * u:(base + Q) * u], Wb3[:])
                # ACT: reduce each 512-slice of the product
                for j in range(n_act):
                    dump = dump_p.tile([P, u], bf16)
                    nc.scalar.activation(
                        dump[:], scrb[:, j * u:(j + 1) * u], Act.Copy,
                        accum_out=e_c[:, q_dve + j:q_dve + j + 1])
            eb_c = ep.tile([P, Q], f32, tag="eb_c")
            nc.vector.tensor_add(eb_c[:], e_c[:], bt[:, base:base + Q])
            th_c = ep.tile([P, Q], f32, tag="th_c")
            nc.scalar.activation(th_c[:], eb_c[:], Act.Tanh)
            p_c = ep.tile([P, Q], f32, tag="p_c")
            nc.scalar.activation(p_c[:], th_c[:], Act.Exp,
                                 accum_out=rs[:, c:c + 1])

            # weighted sum: slice qi -> col group qi%4 so 4 fp32 matmuls
            # stream concurrently; partial rows at psum partitions {0,32,64,96}
            for qi in range(Q):
                q = base + qi
                g = qi % 4
                first = c == 0 and qi < 4
                last = c == NC_CHUNK - 1 and qi >= Q - 4
                nc.tensor.matmul(
                    wsum[32 * g:32 * g + 1, :],
                    p_c[:, qi:qi + 1],
                    xt[:, q * u:(q + 1) * u],
                    start=first, stop=last,
                    tile_position=(0, 32 * g),
                )

            if c == 0 and pending is not None:
                emit_epilogue(*pending)
                pending = None

        pending = (s, wsum, rs)
    emit_epilogue(*pending)


def build_nc(b_loc=B_LOC, t_len=T, u=U, xbufs=5):
    key = (b_loc, t_len, u, xbufs)
    if key in _BUILD_CACHE:
        return _BUILD_CACHE[key]
    import concourse.bacc as bacc
    import concourse.tile as tile
    from concourse import mybir

    nc = bacc.Bacc(
        "TRN2",
        target_bir_lowering=False,
        debug=False,
        num_devices=N_CORES,
    )
    x = nc.dram_tensor("x", [b_loc, t_len, u], mybir.dt.float32, kind="ExternalInput").ap()
    W = nc.dram_tensor("W", [u, 1], mybir.dt.float32, kind="ExternalInput").ap()
    b = nc.dram_tensor("b", [t_len, 1], mybir.dt.float32, kind="ExternalInput").ap()
    y = nc.dram_tensor("y", [b_loc, u], mybir.dt.float32, kind="ExternalOutput").ap()

    with tile.TileContext(nc) as tc:
        with ExitStack() as ctx:
            _emit(ctx, tc, x, W, b, y, xbufs)
    nc.compile()
    _BUILD_CACHE[key] = nc
    return nc


def kernel(x, W, b):
    x = np.ascontiguousarray(np.asarray(x, dtype=np.float32))
    W = np.ascontiguousarray(np.asarray(W, dtype=np.float32))
    b = np.ascontiguousarray(np.asarray(b, dtype=np.float32))
    assert x.shape == (B, T, U), x.shape

    from concourse.bass_utils import run_bass_kernel_spmd

    nc = build_nc()
    in_maps = [
        {
            "x": np.ascontiguousarray(x[i * B_LOC:(i + 1) * B_LOC]),
            "W": W,
            "b": b,
        }
        for i in range(N_CORES)
    ]
    res = run_bass_kernel_spmd(nc, in_maps, core_ids=list(range(N_CORES)))
    return np.concatenate([r["y"] for r in res.results], axis=0)


# revision 24
# speedup vs baseline: 1.0411x; 1.0411x over previous
"""Bass/Tile TRN2 kernel for additive-attention pooling.

Math per sample s:
    e = tanh(x[s] @ W + b)          # (T, 1)
    a = softmax(e, axis=0)          # over T
    y[s] = sum_t a[t] * x[s, t, :]  # (U,)

tanh is bounded in (-1, 1), so softmax needs no max-subtraction:
    p = exp(e);  y[s] = (sum_t p[t] x[s,t]) / (sum_t p[t])

Sharding: data-parallel over batch across 8 NeuronCores (32 samples each).

Per-core dataflow. x is streamed once, in "superchunks" of Q*128 timesteps
laid out q-packed (t = sc*Q*128 + p*Q + q) so each SBUF partition receives
Q*2KiB contiguous from HBM in one large DMA (few DMA-issue instructions,
big packets). Per superchunk:
  - DVE scalar_tensor_tensor: e_col = sum_u (x * W) per 128x512 slice
    (single fused pass over x, 1 elem/lane/cyc)
  - DVE adds bias b; ACT tanh; ACT exp with accum_out row sums
  - PE weighted sum, 4-way column-tiled: slice q -> col group q%4, fp32
    matmuls in distinct col groups stream via separate XBUSes (~4x conc.)
Per sample: partial rows {0,32,64,96} combined via a selector matmul,
denominator via ones-matmul, DVE reciprocal, ACT scaled copy, DMA out.
"""

from contextlib import ExitStack

import numpy as np

B, T, U = 256, 2048, 512
N_CORES = 8
B_LOC = B // N_CORES
P = 128

_BUILD_CACHE = {}


def _emit(ctx, tc, x, W, b, y, xbufs):
    from concourse import mybir

    nc = tc.nc
    f32 = mybir.dt.float32
    Alu = mybir.AluOpType
    Act = mybir.ActivationFunctionType

    b_loc, t_len, u = x.shape
    tch = t_len // P          # 128-timestep chunks
    Q = 8 if tch % 8 == 0 else 4   # chunks per superchunk
    nsc = tch // Q

    const = ctx.enter_context(tc.tile_pool(name="const", bufs=1))
    xp = ctx.enter_context(tc.tile_pool(name="xp", bufs=xbufs))
    scr_p = ctx.enter_context(tc.tile_pool(name="scr", bufs=2))
    ep = ctx.enter_context(tc.tile_pool(name="ep", bufs=4))
    sp = ctx.enter_context(tc.tile_pool(name="sp", bufs=6))
    op = ctx.enter_context(tc.tile_pool(name="op", bufs=4))
    ps_wb = ctx.enter_context(tc.tile_pool(name="ps_wb", bufs=1, space="PSUM"))
    ps_w = ctx.enter_context(tc.tile_pool(name="ps_w", bufs=4, space="PSUM"))
    ps_s = ctx.enter_context(tc.tile_pool(name="ps_s", bufs=2, space="PSUM"))

    # ---- constants ----
    # W as a [1, U] row, broadcast to all 128 partitions via a K=1 matmul.
    w_row = const.tile([1, u], f32)
    nc.sync.dma_start(w_row[:], W.rearrange("u o -> o u"))
    ones_row = const.tile([1, P], f32)
    nc.vector.memset(ones_row[:], 1.0)
    ones_col = const.tile([P, 1], f32)
    nc.vector.memset(ones_col[:], 1.0)
    # selector: 1.0 at partitions {0,32,64,96} — sums the 4 col-group partials
    sel = const.tile([P, 1], f32)
    nc.vector.memset(sel[:], 0.0)
    for j in range(4):
        nc.vector.memset(sel[32 * j:32 * j + 1, :], 1.0)
    wb_ps = ps_wb.tile([P, u], f32, tag="wb_ps")
    nc.tensor.matmul(wb_ps[:], ones_row[:], w_row[:], start=True, stop=True)
    Wb = const.tile([P, u], f32)
    nc.vector.tensor_copy(Wb[:], wb_ps[:])
    # b rearranged to [partition, (sc q)] matching the q-packed x layout
    bt3 = const.tile([P, nsc, Q], f32)
    nc.sync.dma_start(bt3[:], b.rearrange("(sc p q) o -> p sc (q o)", p=P, q=Q))

    xr = x.rearrange("s (sc p q) u -> s sc p (q u)", p=P, q=Q)

    def emit_epilogue(se, wsum, rs):
        # denominator: rstot[p] = sum_sc rs[p,sc]; d = rstot.T @ ones
        rsd = sp.tile([P, nsc], f32, tag="rsd")
        rstot = sp.tile([P, 1], f32, tag="rstot")
        nc.scalar.activation(rsd[:], rs[:], Act.Copy, accum_out=rstot[:])
        s_ps = ps_s.tile([1, 1], f32)
        nc.tensor.matmul(s_ps[:], rstot[:], ones_col[:], start=True, stop=True)
        inv = sp.tile([1, 1], f32, tag="inv")
        nc.vector.reciprocal(inv[:], s_ps[:])

        # combine the 4 partial rows: copy bank to SBUF, then sel.T @ rows
        wsb = op.tile([P, u], f32, tag="wsb")
        nc.scalar.activation(wsb[:], wsum[:], Act.Copy)
        nc.tensor.matmul(wsum[0:1, :], sel[:], wsb[:], start=True, stop=True)

        orow = op.tile([1, u], f32, tag="orow")
        nc.scalar.activation(orow[:], wsum[0:1, :], Act.Copy, scale=inv[:])
        nc.sync.dma_start(y[se:se + 1, :], orow[:])

    # Epilogues are emitted one superchunk late (inside the NEXT sample):
    # DVE/ACT are strict-FIFO, so an epilogue op (reciprocal, PSUM combine)
    # waiting on the PE at the queue head would head-of-line block the next
    # sample's dot-product work. Deferred, its sems are satisfied on arrival.
    pending = None
    for s in range(b_loc):
        wsum = ps_w.tile([P, u], f32, tag="wsum")
        nc.scalar.memzero(wsum[:])
        rs = sp.tile([P, nsc], f32, tag="rs")
        for sc in range(nsc):
            # fat tile: Q*128 timesteps, partition p holds Q contiguous
            # HBM rows -> one Q*256KiB DMA with Q*2KiB packets
            xt = xp.tile([P, Q * u], f32)
            nc.sync.dma_start(xt[:], xr[s, sc])
            e_sc = ep.tile([P, Q], f32, tag="e_sc")
            for q in range(Q):
                scr = scr_p.tile([P, u], f32)
                nc.vector.scalar_tensor_tensor(
                    out=scr[:],
                    in0=xt[:, q * u:(q + 1) * u],
                    scalar=1.0,
                    in1=Wb[:],
                    op0=Alu.mult,
                    op1=Alu.mult,
                    accum_out=e_sc[:, q:q + 1],
                )
            eb_sc = ep.tile([P, Q], f32, tag="eb_sc")
            nc.vector.tensor_add(eb_sc[:], e_sc[:], bt3[:, sc, :])
            th_sc = ep.tile([P, Q], f32, tag="th_sc")
            nc.scalar.activation(th_sc[:], eb_sc[:], Act.Tanh)
            p_sc = ep.tile([P, Q], f32, tag="p_sc")
            nc.scalar.activation(p_sc[:], th_sc[:], Act.Exp,
                                 accum_out=rs[:, sc:sc + 1])
            # weighted sum: slice q -> col group q%4, partial at psum row 32j
            for q in range(Q):
                c = sc * Q + q
                j = q % 4
                nc.tensor.matmul(
                    wsum[32 * j:32 * j + 1, :],
                    p_sc[:, q:q + 1],
                    xt[:, q * u:(q + 1) * u],
                    start=(c < 4), stop=(c >= tch - 4),
                    tile_position=(0, 32 * j),
                )

            if sc == 0 and pending is not None:
                emit_epilogue(*pending)
                pending = None

        pending = (s, wsum, rs)
    emit_epilogue(*pending)


def build_nc(b_loc=B_LOC, t_len=T, u=U, xbufs=10):
    key = (b_loc, t_len, u, xbufs)
    if key in _BUILD_CACHE:
        return _BUILD_CACHE[key]
    import concourse.bacc as bacc
    import concourse.tile as tile
    from concourse import mybir

    nc = bacc.Bacc(
        "TRN2",
        target_bir_lowering=False,
        debug=False,
        num_devices=N_CORES,
    )
    x = nc.dram_tensor("x", [b_loc, t_len, u], mybir.dt.float32, kind="ExternalInput").ap()
    W = nc.dram_tensor("W", [u, 1], mybir.dt.float32, kind="ExternalInput").ap()
    b = nc.dram_tensor("b", [t_len, 1], mybir.dt.float32, kind="ExternalInput").ap()
    y = nc.dram_tensor("y", [b_loc, u], mybir.dt.float32, kind="ExternalOutput").ap()

    with tile.TileContext(nc) as tc:
        with ExitStack() as ctx:
            _emit(ctx, tc, x, W, b, y, xbufs)
    nc.compile()
    _BUILD_CACHE[key] = nc
    return nc


def kernel(x, W, b):
    x = np.ascontiguousarray(np.asarray(x, dtype=np.float32))
    W = np.ascontiguousarray(np.asarray(W, dtype=np.float32))
    b = np.ascontiguousarray(np.asarray(b, dtype=np.float32))
    assert x.shape == (B, T, U), x.shape

    from concourse.bass_utils import run_bass_kernel_spmd

    nc = build_nc()
    in_maps = [
        {
            "x": np.ascontiguousarray(x[i * B_LOC:(i + 1) * B_LOC]),
            "W": W,
            "b": b,
        }
        for i in range(N_CORES)
    ]
    res = run_bass_kernel_spmd(nc, in_maps, core_ids=list(range(N_CORES)))
    return np.concatenate([r["y"] for r in res.results], axis=0)
